# revision 1
# baseline (speedup 1.0000x reference)
"""AttentionBlock (GroupNorm + single-head spatial attention + proj + residual)
on 8 trn2 NeuronCores, data-parallel over the batch (1 image per core).

Full inputs in, full outputs out. Layouts are prepared host-side so every DMA
is contiguous and the device program needs no transposes:
  - activations live as [128 part, ct, pix]  (channel tiles of 128)
  - weights are passed pre-transposed as [c_in part, ct_in, c_out]
  - v is produced directly transposed (v^T = xn^T @ W_v^T) so the
    attention contraction over pixels has pixels on partitions everywhere.

Matmuls run in float32r (single-pass fp32 PE mode, ~0.28us per
128x128x512 tile at 2.4GHz; measured end-to-end rel err vs fp32 ~2e-5).
A burst of throwaway matmuls at kernel start keeps the PE busy through the
input-DMA window so the HAM clock gate reaches 8/8 before the real GEMMs.
"""

import sys

sys.path.insert(0, "/opt/trn_rl_repo")

import numpy as np

import concourse.bass as bass
import concourse.tile as tile
from concourse import bacc, mybir
from concourse.bass_utils import run_bass_kernel_spmd
from concourse.tile_rust import add_dep_helper

F32 = mybir.dt.float32
F32R = mybir.dt.float32r

C = 512          # channels
NPIX = 1024      # pixels per image (32*32)
CT = 4           # channel tiles of 128
JT = 8           # pixel tiles of 128
NH = 2           # halves of NPIX for the 512-wide moving dim
G = 32           # groups
GS = 16          # channels per group
EPS = 1e-5
SCALE = C ** -0.5
WARM_MMS = 38    # PE warm-up matmuls during the input-DMA window

# matmul dtype for the big GEMMs: float32r streams one 512-wide tile in
# ~280ns warm (vs ~1.1us for 2-pass fp32).  Set to F32 for full precision.
MM_DT = F32R

TRACE = False          # set True (from test.py) to capture an NTFF profile
TRACE_KW = {}          # extra kwargs for run_bass_kernel_spmd
LAST_RESULTS = None    # BassKernelResults of the most recent run

_cache = {}


def _build(fold_qk=True):
    nc = bacc.Bacc("TRN2")

    x_d = nc.dram_tensor("x", [128, CT, NPIX], F32, kind="ExternalInput")
    qwcols = 2 * C if fold_qk else 3 * C
    qw_d = nc.dram_tensor("qw", [128, CT, qwcols], MM_DT, kind="ExternalInput")
    pw_d = nc.dram_tensor("pw", [128, CT, C], MM_DT, kind="ExternalInput")
    gnw_d = nc.dram_tensor("gnw", [128, CT], F32, kind="ExternalInput")
    gnb_d = nc.dram_tensor("gnb", [128, CT], F32, kind="ExternalInput")
    if not fold_qk:
        qb_d = nc.dram_tensor("qb", [128, CT], F32, kind="ExternalInput")
        kb_d = nc.dram_tensor("kb", [128, CT], F32, kind="ExternalInput")
    pb_d = nc.dram_tensor("pb", [128, CT], F32, kind="ExternalInput")
    y_d = nc.dram_tensor("y", [128, CT, NPIX], F32, kind="ExternalOutput")

    # Indicator constants for the cross-partition group reductions.
    # ind1[p, ct*G + g] = 1 iff channel (ct*128+p) belongs to group g.
    ind1 = np.zeros((128, CT * G), np.float32)
    for ct in range(CT):
        for p in range(128):
            ind1[p, ct * G + ct * 8 + p // GS] = 1.0
    # ind2[g, c] = 1 iff channel c belongs to group g.
    ind2 = np.zeros((G, C), np.float32)
    for c in range(C):
        ind2[c // GS, c] = 1.0
    ind1_d = nc.inline_tensor(ind1, name="ind1")
    ind2_d = nc.inline_tensor(ind2, name="ind2")
    onesc_d = nc.dram_tensor("onesc", [128, 512], MM_DT, kind="ExternalInput")
    onesr_d = nc.dram_tensor("onesr", [1, 128], MM_DT, kind="ExternalInput")

    with tile.TileContext(nc) as tc:
        with (
            nc.allow_low_precision(reason="float32r matmul operands (4B fp32 bits)"),
            tc.tile_pool(name="persist", bufs=1) as pers,
            tc.tile_pool(name="small", bufs=4) as spool,
            tc.tile_pool(name="ps", bufs=8, space="PSUM") as psp,
        ):
            # ---- warm-up source first, then x gets the DMA bandwidth -------
            onesc_sb = pers.tile([128, 512], MM_DT)
            nc.sync.dma_start(onesc_sb[:], onesc_d[:])
            ones_col = onesc_sb[:, 0:1]

            # ---- x, one DMA per (ct, half) so bn_stats overlaps ------------
            x_sb = pers.tile([128, CT, NPIX], F32)
            x_dmas = []
            for ct in range(CT):
                for nh in range(NH):
                    x_dmas.append(
                        nc.sync.dma_start(
                            x_sb[:, ct, nh * 512 : (nh + 1) * 512],
                            x_d[:, ct, nh * 512 : (nh + 1) * 512],
                        )
                    )

            # ---- tiny loads (after x in the issue queue) -------------------
            gnw_sb = pers.tile([128, CT], F32)
            nc.sync.dma_start(gnw_sb[:], gnw_d[:])
            gnb_sb = pers.tile([128, CT], F32)
            nc.sync.dma_start(gnb_sb[:], gnb_d[:])
            ind1_sb = pers.tile([128, CT * G], F32)
            nc.sync.dma_start(ind1_sb[:], ind1_d[:])
            ind2_sb = pers.tile([G, C], F32)
            nc.sync.dma_start(ind2_sb[:], ind2_d[:])
            if not fold_qk:
                qb_sb = pers.tile([128, CT], F32)
                nc.sync.dma_start(qb_sb[:], qb_d[:])
                kb_sb = pers.tile([128, CT], F32)
                nc.sync.dma_start(kb_sb[:], kb_d[:])
            pb_sb = pers.tile([128, CT], F32)
            nc.sync.dma_start(pb_sb[:], pb_d[:])
            ones_row = pers.tile([1, 128], MM_DT)
            nc.sync.dma_start(ones_row[:], onesr_d[:])

            # ---- weights, serialized behind x so x gets the DMA bandwidth --
            qw_sb = pers.tile([128, CT, qwcols], MM_DT)
            for ci in range(CT):
                d = nc.sync.dma_start(qw_sb[:, ci, :], qw_d[:, ci, :])
                add_dep_helper(d.ins, x_dmas[-1].ins, sync=True,
                               reason="let x DMA finish first")
            pw_sb = pers.tile([128, CT, C], MM_DT)
            d = nc.sync.dma_start(pw_sb[:], pw_d[:])
            add_dep_helper(d.ins, x_dmas[-1].ins, sync=True,
                           reason="let x DMA finish first")

            eps_sb = pers.tile([G, 1], F32)
            nc.vector.memset(eps_sb[:], EPS)
            ones_row32 = pers.tile([1, 128], F32)
            nc.vector.memset(ones_row32[:], 1.0)
            ones_col32 = pers.tile([128, 1], F32)
            nc.vector.memset(ones_col32[:], 1.0)

            # ---- PE warm-up: keep HAM busy while the input DMAs stream -----
            warm_ps = psp.tile([128, 512], F32, tag="ps")
            for _ in range(WARM_MMS):
                nc.tensor.matmul(
                    warm_ps[:], onesc_sb[:, 0:128], onesc_sb[:], start=True, stop=True
                )

            # ---- group norm ------------------------------------------------
            # per-channel mean / E[x^2] along pixels, then group-combine via
            # indicator matmuls (contraction over the partition dim).
            statcols = pers.tile([128, CT, 2], F32)
            for ct in range(CT):
                st6 = spool.tile([128, 2, 6], F32, tag="st6")
                nc.vector.bn_stats(st6[:, 0, :], x_sb[:, ct, 0:512])
                nc.vector.bn_stats(st6[:, 1, :], x_sb[:, ct, 512:1024])
                mv = spool.tile([128, 2], F32, tag="mv")
                nc.vector.bn_aggr(mv[:], st6[:])
                nc.vector.tensor_copy(statcols[:, ct, 0:1], mv[:, 0:1])
                # E[x^2] = var + mean^2
                nc.vector.tensor_mul(statcols[:, ct, 1:2], mv[:, 0:1], mv[:, 0:1])
                nc.vector.tensor_add(
                    statcols[:, ct, 1:2], statcols[:, ct, 1:2], mv[:, 1:2]
                )

            gsum_ps = psp.tile([G, 2], F32, tag="ps")
            for ct in range(CT):
                nc.tensor.matmul(
                    gsum_ps[:],
                    ind1_sb[:, ct * G : (ct + 1) * G],
                    statcols[:, ct, :],
                    start=(ct == 0),
                    stop=(ct == CT - 1),
                )
            gs_sb = spool.tile([G, 2], F32, tag="gs")
            nc.vector.tensor_scalar_mul(gs_sb[:], gsum_ps[:], 1.0 / GS)
            var32 = spool.tile([G, 1], F32, tag="var32")
            nc.vector.tensor_mul(var32[:], gs_sb[:, 0:1], gs_sb[:, 0:1])
            nc.vector.tensor_sub(var32[:], gs_sb[:, 1:2], var32[:])
            # grow = [rstd, mean * rstd] per group
            grow = pers.tile([G, 2], F32)
            nc.scalar.activation(
                grow[:, 0:1],
                var32[:],
                mybir.ActivationFunctionType.Sqrt,
                bias=eps_sb[:],
            )
            nc.vector.reciprocal(grow[:, 0:1], grow[:, 0:1])
            nc.vector.tensor_mul(grow[:, 1:2], gs_sb[:, 0:1], grow[:, 0:1])

            # broadcast group stats back to channels; fold gn weight/bias into
            # per-channel scale A and bias B:  xn = x*A + B
            xn_sb = pers.tile([128, CT, NPIX], MM_DT)
            chsb = pers.tile([128, CT, 2], F32)
            for ct in range(CT):
                bc_ps = psp.tile([128, 2], F32, tag="ps")
                nc.tensor.matmul(
                    bc_ps[:],
                    ind2_sb[:, ct * 128 : (ct + 1) * 128],
                    grow[:],
                    start=True,
                    stop=True,
                )
                nc.vector.tensor_mul(
                    chsb[:, ct, 0:1], gnw_sb[:, ct : ct + 1], bc_ps[:, 0:1]
                )
                nc.vector.tensor_mul(
                    chsb[:, ct, 1:2], gnw_sb[:, ct : ct + 1], bc_ps[:, 1:2]
                )
                nc.vector.tensor_sub(
                    chsb[:, ct, 1:2], gnb_sb[:, ct : ct + 1], chsb[:, ct, 1:2]
                )
                nc.vector.tensor_scalar(
                    out=xn_sb[:, ct, :],
                    in0=x_sb[:, ct, :],
                    scalar1=chsb[:, ct, 0:1],
                    scalar2=chsb[:, ct, 1:2],
                    op0=mybir.AluOpType.mult,
                    op1=mybir.AluOpType.add,
                )

            # ---- queries/keys --------------------------------------------
            # fold path: t = A @ xn with A = W_q^T W_k (host-precomputed);
            # S^T = xn^T t then equals k^T q up to softmax-invariant terms.
            # legacy path: explicit q, k with their biases.
            if fold_qk:
                t_sb = pers.tile([128, CT, NPIX], MM_DT)
                for co in range(CT):
                    for nh in range(NH):
                        ps = psp.tile([128, 512], F32, tag="ps")
                        for ci in range(CT):
                            nc.tensor.matmul(
                                ps[:],
                                qw_sb[:, ci, co * 128 : (co + 1) * 128],
                                xn_sb[:, ci, nh * 512 : (nh + 1) * 512],
                                start=(ci == 0),
                                stop=(ci == CT - 1),
                            )
                        nc.scalar.activation(
                            t_sb[:, co, nh * 512 : (nh + 1) * 512],
                            ps[:],
                            mybir.ActivationFunctionType.Identity,
                        )
                q_sb = t_sb
                k_sb = xn_sb
            else:
                q_sb = pers.tile([128, CT, NPIX], MM_DT)
                k_sb = pers.tile([128, CT, NPIX], MM_DT)
                for dst, wofs, b_sb in ((q_sb, 0, qb_sb), (k_sb, C, kb_sb)):
                    for co in range(CT):
                        for nh in range(NH):
                            ps = psp.tile([128, 512], F32, tag="ps")
                            for ci in range(CT):
                                nc.tensor.matmul(
                                    ps[:],
                                    qw_sb[:, ci, wofs + co * 128 : wofs + (co + 1) * 128],
                                    xn_sb[:, ci, nh * 512 : (nh + 1) * 512],
                                    start=(ci == 0),
                                    stop=(ci == CT - 1),
                                )
                            nc.scalar.activation(
                                dst[:, co, nh * 512 : (nh + 1) * 512],
                                ps[:],
                                mybir.ActivationFunctionType.Identity,
                                bias=b_sb[:, co : co + 1],
                            )

            # ---- v^T = xn^T @ W_v^T + 1 x b_v  (out: [pix part, c_out]) ----
            vt_sb = pers.tile([128, JT, C], MM_DT)
            for jt in range(JT):
                ps = psp.tile([128, 512], F32, tag="ps")
                for ci in range(CT):
                    nc.tensor.matmul(
                        ps[:],
                        xn_sb[:, ci, jt * 128 : (jt + 1) * 128],
                        qw_sb[:, ci, qwcols - C : qwcols],
                        start=(ci == 0),
                        stop=(ci == CT - 1),
                    )
                nc.vector.tensor_copy(vt_sb[:, jt, :], ps[:])

            # ---- S^T = k^T q (pix_j on partitions), E = exp(scale * S^T) ---
            # ih-major: each half's denominator + reciprocal hides under the
            # other half's matmuls.
            e_sb = pers.tile([128, JT, NPIX], MM_DT)
            recip_sb = pers.tile([1, NPIX], F32)
            for nh in range(NH):
                dps = psp.tile([1, 512], F32, name=f"den{nh}", tag="ps")
                for jt in range(JT):
                    ps = psp.tile([128, 512], F32, tag="ps")
                    for ci in range(CT):
                        nc.tensor.matmul(
                            ps[:],
                            k_sb[:, ci, jt * 128 : (jt + 1) * 128],
                            q_sb[:, ci, nh * 512 : (nh + 1) * 512],
                            start=(ci == 0),
                            stop=(ci == CT - 1),
                        )
                    esl = e_sb[:, jt, nh * 512 : (nh + 1) * 512]
                    nc.scalar.activation(
                        esl, ps[:], mybir.ActivationFunctionType.Exp, scale=SCALE
                    )
                    nc.tensor.matmul(
                        dps[:],
                        ones_col,
                        esl,
                        start=(jt == 0),
                        stop=(jt == JT - 1),
                    )
                rsl = recip_sb[0:1, nh * 512 : (nh + 1) * 512]
                rscr = spool.tile([1, 512], F32, tag="rscr")
                nc.vector.reciprocal_approx_accurate(rsl, dps[:], rscr[:])

            # ---- broadcast 1/denom across partitions (fp32 matmul) ---------
            rb_sb = pers.tile([128, NPIX], F32)

            def bcast_recip(nh):
                bp = psp.tile([128, 512], F32, name=f"bp{nh}", tag="ps")
                nc.tensor.matmul(
                    bp[:],
                    ones_row32[0:1, :],
                    recip_sb[0:1, nh * 512 : (nh + 1) * 512],
                    start=True,
                    stop=True,
                )
                nc.scalar.activation(
                    rb_sb[:, nh * 512 : (nh + 1) * 512],
                    bp[:],
                    mybir.ActivationFunctionType.Identity,
                )

            # ---- att = v^T^T @ E, normalized on evacuation -----------------
            att_sb = pers.tile([128, CT, NPIX], MM_DT)
            for nh in range(NH):
                bcast_recip(nh)
                for ct in range(CT):
                    ps = psp.tile([128, 512], F32, tag="ps")
                    for jt in range(JT):
                        nc.tensor.matmul(
                            ps[:],
                            vt_sb[:, jt, ct * 128 : (ct + 1) * 128],
                            e_sb[:, jt, nh * 512 : (nh + 1) * 512],
                            start=(jt == 0),
                            stop=(jt == JT - 1),
                        )
                    nc.vector.tensor_mul(
                        att_sb[:, ct, nh * 512 : (nh + 1) * 512],
                        ps[:],
                        rb_sb[:, nh * 512 : (nh + 1) * 512],
                    )

            # ---- out = proj_w @ att + proj_b + x, streamed to DRAM ---------
            for co in range(CT):
                for nh in range(NH):
                    ps = psp.tile([128, 512], F32, tag="ps")
                    for ci in range(CT):
                        nc.tensor.matmul(
                            ps[:],
                            pw_sb[:, ci, co * 128 : (co + 1) * 128],
                            att_sb[:, ci, nh * 512 : (nh + 1) * 512],
                            start=(ci == 0),
                            stop=(ci == CT - 1),
                        )
                    sl = (slice(None), co, slice(nh * 512, (nh + 1) * 512))
                    nc.vector.scalar_tensor_tensor(
                        out=x_sb[sl],
                        in0=ps[:],
                        scalar=pb_sb[:, co : co + 1],
                        in1=x_sb[sl],
                        op0=mybir.AluOpType.add,
                        op1=mybir.AluOpType.add,
                    )
                    nc.sync.dma_start(y_d[sl], x_sb[sl])

    nc.compile()
    return nc


def kernel(x, gn_weight, gn_bias, qkv_w, qkv_b, proj_w, proj_b):
    global LAST_RESULTS
    b, c, h, w = x.shape
    assert (b, c, h * w) == (8, C, NPIX)

    qkv_b = np.asarray(qkv_b, np.float32)
    qkv_w = np.asarray(qkv_w, np.float32)
    proj_w = np.asarray(proj_w, np.float32)
    # The per-query bias term cancels in softmax; a nonzero q-bias would
    # contribute a per-key term, so only then fall back to explicit q/k.
    fold_qk = not np.any(qkv_b[0:C])

    if ("nc", fold_qk) not in _cache:
        _cache[("nc", fold_qk)] = _build(fold_qk)
    nc = _cache[("nc", fold_qk)]

    def col(v):  # [512] vector -> [128, CT] per-partition columns
        return np.ascontiguousarray(np.asarray(v, np.float32).reshape(CT, 128).T)

    def wtile(wT, cols):  # [c_in, cols] -> [128, CT, cols]
        return np.ascontiguousarray(
            np.asarray(wT, np.float32).reshape(CT, 128, cols).transpose(1, 0, 2)
        )

    if fold_qk:
        # A^T = W_q^T W_k in fp64, laid out like a weight: lhsT[b, a]
        At = (qkv_w[0:C].astype(np.float64).T @ qkv_w[C : 2 * C].astype(np.float64))
        qw_host = np.concatenate(
            [At.astype(np.float32), qkv_w[2 * C :].T], axis=1
        )  # [c_in, 2C]
    else:
        qw_host = qkv_w.T  # [c_in, 3C]

    shared = {
        "qw": wtile(qw_host, qw_host.shape[1]),
        "pw": wtile(proj_w.T, C),
        "gnw": col(gn_weight),
        "gnb": col(gn_bias),
        # attention rows sum to 1, so att(v + b_v) = att(v) + b_v; fold the
        # v bias through proj into the proj bias on the host.
        "pb": col(proj_b + proj_w @ qkv_b[2 * C :]),
        "onesc": np.ones((128, 512), np.float32),
        "onesr": np.ones((1, 128), np.float32),
    }
    if not fold_qk:
        shared["qb"] = col(qkv_b[0:C])
        shared["kb"] = col(qkv_b[C : 2 * C])
    xs = np.asarray(x, np.float32).reshape(b, CT, 128, NPIX)
    in_maps = [
        {"x": np.ascontiguousarray(xs[i].transpose(1, 0, 2)), **shared}
        for i in range(b)
    ]

    res = run_bass_kernel_spmd(
        nc, in_maps, core_ids=list(range(8)), trace=TRACE, **TRACE_KW
    )
    LAST_RESULTS = res
    out = np.stack(
        [r["y"].transpose(1, 0, 2).reshape(c, h, w) for r in res.results]
    )
    return out.astype(np.float32)



# revision 6
# speedup vs baseline: 1.3139x; 1.3139x over previous
"""AttentionBlock (GroupNorm + single-head spatial attention + proj + residual)
on 8 trn2 NeuronCores, data-parallel over the batch (1 image per core).

v2 design (vs v1 baseline at ~94us):
  - proj_w is folded into W_v host-side (attention output is linear in v):
    the proj GEMM stage disappears entirely.
  - q/k fold as in v1: t = M^T xn with M = Wq^T Wk precomputed host-side,
    E = exp(scale * xn^T t) == exp(scale * S).
  - All four big GEMMs (t, v~, S, att) run in fp8 e4m3 with DoubleRow perf
    mode (2 k-subtiles of 128 per instruction, 0.5 PE cycles/row).  PSUM
    accumulation stays fp32.  Tolerance is 2e-2; measured fp8 error ~5e-3.
  - Attention output is computed TRANSPOSED: att^T[i, c] (pixels on
    partitions) via lhsT = E-slices.  The softmax denominator then lands in
    a [128,1] psum per pixel tile (tiny 1-wide matmuls) and the reciprocal
    is a per-partition scalar in the final evacuation - no broadcast
    matmuls, no [1,512] reciprocals.
  - The residual + proj bias arrives pre-added host-side in the transposed
    layout (xpb = x^T + pb), DMA'd off the critical path; final evacuation
    is a single scalar_tensor_tensor per pixel tile.
  - GroupNorm is pipelined per channel-tile: groups (16 ch) never span the
    128-partition tiles, so each tile's stats -> scale/bias -> xn complete
    as its DMA lands and the GEMMs start ~10us earlier.
"""

import sys

sys.path.insert(0, "/opt/trn_rl_repo")

import numpy as np

import concourse.bass as bass
import concourse.tile as tile
from concourse import bacc, mybir
from concourse.bass_utils import run_bass_kernel_spmd
from concourse.tile_rust import add_dep_helper

F32 = mybir.dt.float32
FP8 = mybir.dt.float8e4  # e4m3
DR = mybir.MatmulPerfMode.DoubleRow

C = 512          # channels
NPIX = 1024      # pixels per image (32*32)
CT = 4           # channel tiles of 128
JT = 8           # pixel tiles of 128
NH = 2           # halves of NPIX for the 512-wide moving dim
G = 32           # groups
GS = 16          # channels per group
GPT = 8          # groups per channel tile (128/16)
EPS = 1e-5
SCALE = C ** -0.5
WARM_MMS = 10    # PE warm-up matmuls (f32, 2-pass) during the input-DMA window

TRACE = False          # set True (from test.py) to capture an NTFF profile
TRACE_KW = {}          # extra kwargs for run_bass_kernel_spmd
LAST_RESULTS = None    # BassKernelResults of the most recent run

_cache = {}


def _build(with_qbias=False):
    nc = bacc.Bacc("TRN2")

    x_d = nc.dram_tensor("x", [128, CT, NPIX], F32, kind="ExternalInput")
    xpb_d = nc.dram_tensor("xpb", [128, JT, C], F32, kind="ExternalInput")
    qa_d = nc.dram_tensor("qa", [128, CT, C], FP8, kind="ExternalInput")
    vw_d = nc.dram_tensor("vw", [128, CT, C], FP8, kind="ExternalInput")
    gnw_d = nc.dram_tensor("gnw", [128, CT], F32, kind="ExternalInput")
    gnb_d = nc.dram_tensor("gnb", [128, CT], F32, kind="ExternalInput")
    if with_qbias:
        rw_d = nc.dram_tensor("rw", [128, CT, 1], FP8, kind="ExternalInput")
    y_d = nc.dram_tensor("y", [128, JT, C], F32, kind="ExternalOutput")

    # Group indicators: within every 128-channel tile the 8 groups are the
    # consecutive 16-channel blocks, identically for each tile.
    # ind1[p, g] = 1/GS if p//16 == g  (group mean / mean-of-squares reduce)
    ind1 = np.zeros((128, GPT), np.float32)
    for p in range(128):
        ind1[p, p // GS] = 1.0 / GS
    # ind2[g, p] = 1 if p//16 == g  (broadcast group stats back to channels)
    ind2 = np.zeros((GPT, 128), np.float32)
    for p in range(128):
        ind2[p // GS, p] = 1.0
    ind1_d = nc.inline_tensor(ind1, name="ind1")
    ind2_d = nc.inline_tensor(ind2, name="ind2")

    with tile.TileContext(nc) as tc:
        with (
            nc.allow_low_precision(reason="fp8 attention path, tol 2e-2"),
            tc.tile_pool(name="persist", bufs=1) as pers,
            tc.tile_pool(name="small", bufs=4) as spool,
            tc.tile_pool(name="bigps", bufs=6, space="PSUM") as bigp,
            tc.tile_pool(name="smallps", bufs=2, space="PSUM") as smp,
        ):
            # ---- constants (no DMA needed) ---------------------------------
            onesc = pers.tile([128, 512], F32)
            nc.vector.memset(onesc[:], 1.0)
            ones2 = pers.tile([128, 2, 1], FP8)
            nc.vector.memset(ones2[:], 1.0)
            eps8 = pers.tile([GPT, 1], F32)
            nc.vector.memset(eps8[:], EPS)

            # ---- x, one DMA per (ct, half) so group norm pipelines ---------
            x_sb = pers.tile([128, CT, NPIX], F32)
            x_dmas = []
            for ct in range(CT):
                for nh in range(NH):
                    x_dmas.append(
                        nc.sync.dma_start(
                            x_sb[:, ct, nh * 512 : (nh + 1) * 512],
                            x_d[:, ct, nh * 512 : (nh + 1) * 512],
                        )
                    )

            # ---- tiny loads ------------------------------------------------
            gnw_sb = pers.tile([128, CT], F32)
            nc.sync.dma_start(gnw_sb[:], gnw_d[:])
            gnb_sb = pers.tile([128, CT], F32)
            nc.sync.dma_start(gnb_sb[:], gnb_d[:])
            ind1_sb = pers.tile([128, GPT], F32)
            nc.sync.dma_start(ind1_sb[:], ind1_d[:])
            ind2_sb = pers.tile([GPT, 128], F32)
            nc.sync.dma_start(ind2_sb[:], ind2_d[:])

            # ---- weights (fp8: 256KB each), serialized behind x ------------
            qa_sb = pers.tile([128, CT, C], FP8)
            d = nc.sync.dma_start(qa_sb[:], qa_d[:])
            add_dep_helper(d.ins, x_dmas[-1].ins, sync=True,
                           reason="x first on the DMA rings")
            vw_sb = pers.tile([128, CT, C], FP8)
            dvw = nc.sync.dma_start(vw_sb[:], vw_d[:])
            add_dep_helper(dvw.ins, x_dmas[-1].ins, sync=True,
                           reason="x first on the DMA rings")
            if with_qbias:
                rw_sb = pers.tile([128, CT, 1], FP8)
                d = nc.sync.dma_start(rw_sb[:], rw_d[:])
                add_dep_helper(d.ins, x_dmas[-1].ins, sync=True,
                               reason="x first on the DMA rings")

            # ---- residual (+proj bias), transposed; needed only at the end -
            xpb_sb = pers.tile([128, JT, C], F32)
            for half in range(4):
                d = nc.sync.dma_start(
                    xpb_sb[:, 2 * half : 2 * half + 2, :],
                    xpb_d[:, 2 * half : 2 * half + 2, :],
                )
                add_dep_helper(d.ins, dvw.ins, sync=True,
                               reason="weights first on the DMA rings")

            # ---- PE warm-up: ramp the PE clock while inputs stream ---------
            warm_ps = bigp.tile([128, 512], F32, tag="ps")
            for _ in range(WARM_MMS):
                nc.tensor.matmul(
                    warm_ps[:], onesc[:, 0:128], onesc[:], start=True, stop=True
                )

            # ---- group norm, pipelined per channel tile --------------------
            xn_sb = pers.tile([128, CT, NPIX], FP8)
            chA = pers.tile([128, CT], F32)
            chB = pers.tile([128, CT], F32)
            for ct in range(CT):
                st6 = spool.tile([128, 2, 6], F32, tag="st6")
                nc.vector.bn_stats(st6[:, 0, :], x_sb[:, ct, 0:512])
                nc.vector.bn_stats(st6[:, 1, :], x_sb[:, ct, 512:1024])
                mv = spool.tile([128, 2], F32, tag="mv")
                nc.vector.bn_aggr(mv[:], st6[:])
                # statcols = [mean, E[x^2]] per channel
                statc = spool.tile([128, 2], F32, tag="statc")
                nc.vector.tensor_copy(statc[:, 0:1], mv[:, 0:1])
                nc.vector.tensor_mul(statc[:, 1:2], mv[:, 0:1], mv[:, 0:1])
                nc.vector.tensor_add(statc[:, 1:2], statc[:, 1:2], mv[:, 1:2])
                # group-combine: [GPT, 2] = (1/GS) * ind1^T @ statcols
                gsp = smp.tile([GPT, 2], F32, tag="sps")
                nc.tensor.matmul(gsp[:], ind1_sb[:], statc[:], start=True, stop=True)
                # gvar = E[x^2] - mean^2 ; grow = [rstd, -mean*rstd]
                gs = spool.tile([GPT, 2], F32, tag="gs")
                nc.vector.tensor_copy(gs[:], gsp[:])
                gvar = spool.tile([GPT, 1], F32, tag="gvar")
                nc.vector.tensor_mul(gvar[:], gs[:, 0:1], gs[:, 0:1])
                nc.vector.tensor_sub(gvar[:], gs[:, 1:2], gvar[:])
                grow = spool.tile([GPT, 2], F32, tag="grow")
                nc.scalar.activation(
                    grow[:, 0:1], gvar[:],
                    mybir.ActivationFunctionType.Sqrt, bias=eps8[:],
                )
                nc.vector.reciprocal(grow[:, 0:1], grow[:, 0:1])
                nc.vector.scalar_tensor_tensor(
                    out=grow[:, 1:2], in0=gs[:, 0:1], scalar=-1.0,
                    in1=grow[:, 0:1],
                    op0=mybir.AluOpType.mult, op1=mybir.AluOpType.mult,
                )
                # broadcast to channels; fold gn weight/bias:  xn = x*A + B
                bcp = smp.tile([128, 2], F32, tag="sps")
                nc.tensor.matmul(bcp[:], ind2_sb[:], grow[:], start=True, stop=True)
                nc.vector.tensor_scalar(
                    out=chA[:, ct : ct + 1], in0=bcp[:, 0:1],
                    scalar1=gnw_sb[:, ct : ct + 1], scalar2=None,
                    op0=mybir.AluOpType.mult,
                )
                nc.vector.scalar_tensor_tensor(
                    out=chB[:, ct : ct + 1], in0=bcp[:, 1:2],
                    scalar=gnw_sb[:, ct : ct + 1], in1=gnb_sb[:, ct : ct + 1],
                    op0=mybir.AluOpType.mult, op1=mybir.AluOpType.add,
                )
                for nh in range(NH):
                    # sbuf->sbuf, so this can run on the otherwise-idle gpsimd
                    nc.gpsimd.tensor_scalar(
                        out=xn_sb[:, ct, nh * 512 : (nh + 1) * 512],
                        in0=x_sb[:, ct, nh * 512 : (nh + 1) * 512],
                        scalar1=chA[:, ct : ct + 1],
                        scalar2=chB[:, ct : ct + 1],
                        op0=mybir.AluOpType.mult,
                        op1=mybir.AluOpType.add,
                    )

            # ---- t = M^T xn  (fp8 DoubleRow, 2 k-pair matmuls per psum) ----
            t_sb = pers.tile([128, CT, NPIX], FP8)
            for co in range(CT):
                for nh in range(NH):
                    ps = bigp.tile([128, 512], F32, tag="ps")
                    for k in range(2):
                        nc.tensor.matmul(
                            ps[:],
                            qa_sb[:, 2 * k : 2 * k + 2, co * 128 : (co + 1) * 128],
                            xn_sb[:, 2 * k : 2 * k + 2, nh * 512 : (nh + 1) * 512],
                            start=(k == 0), stop=(k == 1), perf_mode=DR,
                        )
                    nc.scalar.activation(
                        t_sb[:, co, nh * 512 : (nh + 1) * 512], ps[:],
                        mybir.ActivationFunctionType.Identity,
                    )

            # ---- v~^T = xn^T (proj_w W_v)^T  (out: [pix part, c]) ----------
            vt_sb = pers.tile([128, JT, C], FP8)
            for jt in range(JT):
                ps = bigp.tile([128, 512], F32, tag="ps")
                for k in range(2):
                    nc.tensor.matmul(
                        ps[:],
                        xn_sb[:, 2 * k : 2 * k + 2, jt * 128 : (jt + 1) * 128],
                        vw_sb[:, 2 * k : 2 * k + 2, :],
                        start=(k == 0), stop=(k == 1), perf_mode=DR,
                    )
                nc.vector.tensor_copy(vt_sb[:, jt, :], ps[:])

            # ---- r[j] = scale * bq . k_j  (only when q-bias nonzero) -------
            if with_qbias:
                r_sb = pers.tile([128, JT], F32)
                for jt in range(JT):
                    rp = smp.tile([128, 1], F32, tag="sps")
                    for k in range(2):
                        nc.tensor.matmul(
                            rp[:],
                            xn_sb[:, 2 * k : 2 * k + 2, jt * 128 : (jt + 1) * 128],
                            rw_sb[:, 2 * k : 2 * k + 2, :],
                            start=(k == 0), stop=(k == 1), perf_mode=DR,
                        )
                    nc.vector.tensor_copy(r_sb[:, jt : jt + 1], rp[:])

            # ---- E[j, i] = exp(scale * S[i, j])  (pix_j on partitions) -----
            e_sb = pers.tile([128, JT, NPIX], FP8)
            for nh in range(NH):
                for jt in range(JT):
                    ps = bigp.tile([128, 512], F32, tag="ps")
                    for k in range(2):
                        nc.tensor.matmul(
                            ps[:],
                            xn_sb[:, 2 * k : 2 * k + 2, jt * 128 : (jt + 1) * 128],
                            t_sb[:, 2 * k : 2 * k + 2, nh * 512 : (nh + 1) * 512],
                            start=(k == 0), stop=(k == 1), perf_mode=DR,
                        )
                    bias = r_sb[:, jt : jt + 1] if with_qbias else 0.0
                    nc.scalar.activation(
                        e_sb[:, jt, nh * 512 : (nh + 1) * 512], ps[:],
                        mybir.ActivationFunctionType.Exp,
                        scale=SCALE, bias=bias,
                    )

            # ---- att^T[i, c] = sum_j E[j, i] v~^T[j, c], denominators as
            # [128,1] psums, final evac = (ps * 1/D) + (x^T + pb), streamed --
            rc_sb = pers.tile([128, JT], F32)
            for jt in range(JT):
                dps = smp.tile([128, 1], F32, tag="sps")
                for k in range(4):
                    nc.tensor.matmul(
                        dps[:],
                        e_sb[:, 2 * k : 2 * k + 2, jt * 128 : (jt + 1) * 128],
                        ones2[:],
                        start=(k == 0), stop=(k == 3), perf_mode=DR,
                    )
                nc.vector.reciprocal(rc_sb[:, jt : jt + 1], dps[:])
                ps = bigp.tile([128, 512], F32, tag="ps")
                for k in range(4):
                    nc.tensor.matmul(
                        ps[:],
                        e_sb[:, 2 * k : 2 * k + 2, jt * 128 : (jt + 1) * 128],
                        vt_sb[:, 2 * k : 2 * k + 2, :],
                        start=(k == 0), stop=(k == 3), perf_mode=DR,
                    )
                nc.vector.scalar_tensor_tensor(
                    out=xpb_sb[:, jt, :], in0=ps[:],
                    scalar=rc_sb[:, jt : jt + 1], in1=xpb_sb[:, jt, :],
                    op0=mybir.AluOpType.mult, op1=mybir.AluOpType.add,
                )
                nc.sync.dma_start(y_d[:, jt, :], xpb_sb[:, jt, :])

    nc.compile()
    return nc


def kernel(x, gn_weight, gn_bias, qkv_w, qkv_b, proj_w, proj_b):
    global LAST_RESULTS
    b, c, h, w = x.shape
    assert (b, c, h * w) == (8, C, NPIX)

    f8np = mybir.dt.np(FP8)
    x = np.asarray(x, np.float32)
    qkv_b = np.asarray(qkv_b, np.float32)
    qkv_w = np.asarray(qkv_w, np.float32)
    proj_w = np.asarray(proj_w, np.float32)
    # A nonzero q-bias contributes a per-key softmax term r[j] = bq.k_j;
    # k-bias and v-bias fold away (softmax shift invariance / rows sum to 1).
    with_qbias = bool(np.any(qkv_b[0:C]))

    if ("nc", with_qbias) not in _cache:
        _cache[("nc", with_qbias)] = _build(with_qbias)
    nc = _cache[("nc", with_qbias)]

    def col(v):  # [512] vector -> [128, CT] per-partition columns
        return np.ascontiguousarray(np.asarray(v, np.float32).reshape(CT, 128).T)

    def wtile(wT):  # [c_in, cols] -> [128, CT, cols] fp8
        return np.ascontiguousarray(
            np.asarray(wT).reshape(CT, 128, -1).transpose(1, 0, 2).astype(f8np)
        )

    Wq, Wk, Wv = qkv_w[0:C], qkv_w[C : 2 * C], qkv_w[2 * C :]
    M = Wq.astype(np.float64).T @ Wk.astype(np.float64)        # [c_in, c_out]
    WtT = (proj_w.astype(np.float64) @ Wv.astype(np.float64)).T  # [c_in, c_out]
    pb_eff = proj_b + proj_w @ qkv_b[2 * C :]

    shared = {
        "qa": wtile(M),
        "vw": wtile(WtT),
        "gnw": col(gn_weight),
        "gnb": col(gn_bias),
    }
    if with_qbias:
        rw = SCALE * (Wk.astype(np.float64).T @ qkv_b[0:C].astype(np.float64))
        shared["rw"] = np.ascontiguousarray(
            rw.reshape(CT, 128, 1).transpose(1, 0, 2).astype(f8np)
        )

    xs = x.reshape(b, CT, 128, NPIX)
    xt = x.reshape(b, C, NPIX).transpose(0, 2, 1)  # [b, pix, c]
    in_maps = [
        {
            "x": np.ascontiguousarray(xs[i].transpose(1, 0, 2)),
            "xpb": np.ascontiguousarray(
                (xt[i] + pb_eff).reshape(JT, 128, C).transpose(1, 0, 2)
            ).astype(np.float32),
            **shared,
        }
        for i in range(b)
    ]

    res = run_bass_kernel_spmd(
        nc, in_maps, core_ids=list(range(8)), trace=TRACE, **TRACE_KW
    )
    LAST_RESULTS = res
    out = np.stack(
        [
            r["y"].transpose(1, 0, 2).reshape(NPIX, C).T.reshape(c, h, w)
            for r in res.results
        ]
    )
    return np.ascontiguousarray(out).astype(np.float32)


# revision 9
# speedup vs baseline: 1.6452x; 1.2522x over previous
"""AttentionBlock (GroupNorm + single-head spatial attention + proj + residual)
on 8 trn2 NeuronCores, data-parallel over the batch (1 image per core).

v3 design (v1 baseline ~94us, v2 ~71us):
  - proj_w folded into W_v host-side; q/k folded (t = M^T xn, M = Wq^T Wk).
  - All four big GEMMs (t, S, v~, att) in fp8 e4m3 DoubleRow (2x PE rate,
    measured 259ns per [K=256]x128x512 at full clock).  PSUM stays fp32.
  - Attention output computed transposed (att^T[i,c], pixels on partitions):
    softmax denominators are [128,1] psums from 1-wide matmuls, reciprocal
    is a per-partition scalar in the final evac; residual + proj bias come
    pre-added host-side (xpb = x^T + pb, f32, DMA'd off the critical path).
  - x streams in bf16 (stats + xn only; the residual uses exact f32 xpb),
    halving the critical input DMA.
  - GroupNorm: per-tile stats stream with the DMA; the group-combine chain
    runs once, breadth-first, on [8, CT, 2]-packed tiles.
  - Warm-up: narrow (64-wide) f32r matmuls trickled through the GroupNorm
    phase keep the HAM clock gate open without head-of-line-blocking the
    real matmuls (the v2 mistake: 10 fat f32 warm-ups at cold clock delayed
    the gn matmuls by 7us and the idle gap then half-clocked the next 12us
    HAM window).
"""

import sys

sys.path.insert(0, "/opt/trn_rl_repo")

import numpy as np

import concourse.bass as bass
import concourse.tile as tile
from concourse import bacc, mybir
from concourse.bass_utils import run_bass_kernel_spmd
from concourse.tile_rust import add_dep_helper

F32 = mybir.dt.float32
F32R = mybir.dt.float32r
BF16 = mybir.dt.bfloat16
FP8 = mybir.dt.float8e4  # e4m3
DR = mybir.MatmulPerfMode.DoubleRow

C = 512          # channels
NPIX = 1024      # pixels per image (32*32)
CT = 4           # channel tiles of 128
JT = 8           # pixel tiles of 128
NH = 2           # halves of NPIX for the 512-wide moving dim
G = 32           # groups
GS = 16          # channels per group
GPT = 8          # groups per channel tile (128/16)
EPS = 1e-5
SCALE = C ** -0.5
WARM0 = 14       # narrow warm-up matmuls at kernel start
WARMI = 6        # narrow warm-up matmuls after each channel tile's stats

TRACE = False          # set True (from test.py) to capture an NTFF profile
TRACE_KW = {}          # extra kwargs for run_bass_kernel_spmd
LAST_RESULTS = None    # BassKernelResults of the most recent run

_cache = {}


def _build(with_qbias=False):
    nc = bacc.Bacc("TRN2")

    x_d = nc.dram_tensor("x", [128, CT, NPIX], BF16, kind="ExternalInput")
    xpb_d = nc.dram_tensor("xpb", [128, JT, C], F32, kind="ExternalInput")
    qa_d = nc.dram_tensor("qa", [128, CT, C], FP8, kind="ExternalInput")
    vw_d = nc.dram_tensor("vw", [128, CT, C], FP8, kind="ExternalInput")
    gnw_d = nc.dram_tensor("gnw", [128, CT], F32, kind="ExternalInput")
    gnb_d = nc.dram_tensor("gnb", [128, CT], F32, kind="ExternalInput")
    if with_qbias:
        rw_d = nc.dram_tensor("rw", [128, CT, 1], FP8, kind="ExternalInput")
    y_d = nc.dram_tensor("y", [128, JT, C], F32, kind="ExternalOutput")

    # Group indicators: within every 128-channel tile the 8 groups are the
    # consecutive 16-channel blocks, identically for each tile.
    ind1 = np.zeros((128, GPT), np.float32)   # group reduce (pre-scaled 1/GS)
    for p in range(128):
        ind1[p, p // GS] = 1.0 / GS
    ind2 = np.zeros((GPT, 128), np.float32)   # broadcast back to channels
    for p in range(128):
        ind2[p // GS, p] = 1.0
    ind1_d = nc.inline_tensor(ind1, name="ind1")
    ind2_d = nc.inline_tensor(ind2, name="ind2")

    with tile.TileContext(nc) as tc:
        with (
            nc.allow_low_precision(reason="fp8 attention path, tol 2e-2"),
            tc.tile_pool(name="persist", bufs=1) as pers,
            tc.tile_pool(name="small", bufs=4) as spool,
            tc.tile_pool(name="bigps", bufs=5, space="PSUM") as bigp,
            tc.tile_pool(name="smallps", bufs=3, space="PSUM") as smp,
        ):
            # ---- constants (no DMA needed) ---------------------------------
            onesc = pers.tile([128, 128], F32)
            nc.vector.memset(onesc[:], 1.0)
            ones2 = pers.tile([128, 2, 1], FP8)
            nc.vector.memset(ones2[:], 1.0)
            eps8 = pers.tile([GPT, 1], F32)
            nc.vector.memset(eps8[:], EPS)

            warm_ps = bigp.tile([128, 512], F32, tag="ps")

            def warm(n):
                for _ in range(n):
                    nc.tensor.matmul(
                        warm_ps[:, 0:64],
                        onesc[:].bitcast(F32R),
                        onesc[:, 0:64].bitcast(F32R),
                        start=True, stop=True,
                    )

            # ---- x (bf16), one DMA per (ct, half); stats stream ------------
            x_sb = pers.tile([128, CT, NPIX], BF16)
            x_dmas = []
            for ct in range(CT):
                for nh in range(NH):
                    x_dmas.append(
                        nc.sync.dma_start(
                            x_sb[:, ct, nh * 512 : (nh + 1) * 512],
                            x_d[:, ct, nh * 512 : (nh + 1) * 512],
                        )
                    )

            # ---- tiny loads ------------------------------------------------
            gnw_sb = pers.tile([128, CT], F32)
            nc.sync.dma_start(gnw_sb[:], gnw_d[:])
            gnb_sb = pers.tile([128, CT], F32)
            nc.sync.dma_start(gnb_sb[:], gnb_d[:])
            ind1_sb = pers.tile([128, GPT], F32)
            nc.sync.dma_start(ind1_sb[:], ind1_d[:])
            ind2_sb = pers.tile([GPT, 128], F32)
            nc.sync.dma_start(ind2_sb[:], ind2_d[:])

            # ---- weights (fp8: 256KB each), serialized behind x ------------
            qa_sb = pers.tile([128, CT, C], FP8)
            d = nc.sync.dma_start(qa_sb[:], qa_d[:])
            add_dep_helper(d.ins, x_dmas[-1].ins, sync=True,
                           reason="x first on the DMA rings")
            vw_sb = pers.tile([128, CT, C], FP8)
            dvw = nc.sync.dma_start(vw_sb[:], vw_d[:])
            add_dep_helper(dvw.ins, x_dmas[-1].ins, sync=True,
                           reason="x first on the DMA rings")
            if with_qbias:
                rw_sb = pers.tile([128, CT, 1], FP8)
                d = nc.sync.dma_start(rw_sb[:], rw_d[:])
                add_dep_helper(d.ins, x_dmas[-1].ins, sync=True,
                               reason="x first on the DMA rings")

            # ---- residual (+proj bias), transposed; needed only at the end -
            xpb_sb = pers.tile([128, JT, C], F32)
            for half in range(4):
                d = nc.sync.dma_start(
                    xpb_sb[:, 2 * half : 2 * half + 2, :],
                    xpb_d[:, 2 * half : 2 * half + 2, :],
                )
                add_dep_helper(d.ins, dvw.ins, sync=True,
                               reason="weights first on the DMA rings")

            warm(WARM0)

            # ---- group norm: per-tile stats stream with the DMA ------------
            statc = pers.tile([128, CT, 2], F32)
            for ct in range(CT):
                st6 = spool.tile([128, 2, 6], F32, tag="st6")
                nc.vector.bn_stats(st6[:, 0, :], x_sb[:, ct, 0:512])
                nc.vector.bn_stats(st6[:, 1, :], x_sb[:, ct, 512:1024])
                mv = spool.tile([128, 2], F32, tag="mv")
                nc.vector.bn_aggr(mv[:], st6[:])
                # statc = [mean, E[x^2]] per channel (sbuf->sbuf: gpsimd)
                nc.gpsimd.tensor_copy(statc[:, ct, 0:1], mv[:, 0:1])
                nc.gpsimd.tensor_mul(statc[:, ct, 1:2], mv[:, 0:1], mv[:, 0:1])
                nc.gpsimd.tensor_add(statc[:, ct, 1:2], statc[:, ct, 1:2], mv[:, 1:2])
                warm(WARMI)

            # ---- group-combine, breadth-first over all 32 groups -----------
            gsal = smp.tile([GPT, CT, 2], F32, tag="sps")
            for ct in range(CT):
                nc.tensor.matmul(
                    gsal[:, ct, :], ind1_sb[:], statc[:, ct, :],
                    start=True, stop=True,
                )
            gs_all = spool.tile([GPT, CT, 2], F32, tag="gs")
            nc.vector.tensor_copy(gs_all[:], gsal[:])
            gvar = spool.tile([GPT, CT], F32, tag="gvar")
            nc.vector.tensor_mul(gvar[:], gs_all[:, :, 0], gs_all[:, :, 0])
            nc.vector.tensor_sub(gvar[:], gs_all[:, :, 1], gvar[:])
            grow = spool.tile([GPT, CT, 2], F32, tag="grow")
            gstd = spool.tile([GPT, CT], F32, tag="gstd")
            nc.scalar.activation(
                gstd[:], gvar[:], mybir.ActivationFunctionType.Sqrt, bias=eps8[:],
            )
            nc.vector.reciprocal(grow[:, :, 0], gstd[:])
            nc.vector.scalar_tensor_tensor(
                out=grow[:, :, 1], in0=gs_all[:, :, 0], scalar=-1.0,
                in1=grow[:, :, 0],
                op0=mybir.AluOpType.mult, op1=mybir.AluOpType.mult,
            )
            # broadcast to channels; fold gn weight/bias:  xn = x*A + B
            xn_sb = pers.tile([128, CT, NPIX], FP8)
            chA = pers.tile([128, CT], F32)
            chB = pers.tile([128, CT], F32)
            for ct in range(CT):
                bcp = smp.tile([128, 2], F32, tag="sps")
                nc.tensor.matmul(
                    bcp[:], ind2_sb[:], grow[:, ct, :], start=True, stop=True
                )
                nc.vector.tensor_scalar(
                    out=chA[:, ct : ct + 1], in0=bcp[:, 0:1],
                    scalar1=gnw_sb[:, ct : ct + 1], scalar2=None,
                    op0=mybir.AluOpType.mult,
                )
                nc.vector.scalar_tensor_tensor(
                    out=chB[:, ct : ct + 1], in0=bcp[:, 1:2],
                    scalar=gnw_sb[:, ct : ct + 1], in1=gnb_sb[:, ct : ct + 1],
                    op0=mybir.AluOpType.mult, op1=mybir.AluOpType.add,
                )
                # nh0 on gpsimd, nh1 on vector: both halves land in parallel
                for nh, eng in ((0, nc.gpsimd), (1, nc.vector)):
                    eng.tensor_scalar(
                        out=xn_sb[:, ct, nh * 512 : (nh + 1) * 512],
                        in0=x_sb[:, ct, nh * 512 : (nh + 1) * 512],
                        scalar1=chA[:, ct : ct + 1],
                        scalar2=chB[:, ct : ct + 1],
                        op0=mybir.AluOpType.mult,
                        op1=mybir.AluOpType.add,
                    )

            # ---- t = M^T xn  (fp8 DoubleRow), nh-major for early S ---------
            t_sb = pers.tile([128, CT, NPIX], FP8)
            for nh in range(NH):
                for co in range(CT):
                    ps = bigp.tile([128, 512], F32, tag="ps")
                    for k in range(2):
                        nc.tensor.matmul(
                            ps[:],
                            qa_sb[:, 2 * k : 2 * k + 2, co * 128 : (co + 1) * 128],
                            xn_sb[:, 2 * k : 2 * k + 2, nh * 512 : (nh + 1) * 512],
                            start=(k == 0), stop=(k == 1), perf_mode=DR,
                        )
                    nc.scalar.activation(
                        t_sb[:, co, nh * 512 : (nh + 1) * 512], ps[:],
                        mybir.ActivationFunctionType.Identity,
                    )

            # ---- r[j] = scale * bq . k_j  (only when q-bias nonzero) -------
            if with_qbias:
                r_sb = pers.tile([128, JT], F32)
                for jt in range(JT):
                    rp = smp.tile([128, 1], F32, tag="sps")
                    for k in range(2):
                        nc.tensor.matmul(
                            rp[:],
                            xn_sb[:, 2 * k : 2 * k + 2, jt * 128 : (jt + 1) * 128],
                            rw_sb[:, 2 * k : 2 * k + 2, :],
                            start=(k == 0), stop=(k == 1), perf_mode=DR,
                        )
                    nc.vector.tensor_copy(r_sb[:, jt : jt + 1], rp[:])

            # ---- E[j, i] = exp(scale * S[i, j]); v~ between the nh halves --
            # PE order: S(nh0) -> v~ -> S(nh1) -> att.  The v~ matmuls fill
            # the PE while the scalar engine works through the nh0 exps.
            e_sb = pers.tile([128, JT, NPIX], FP8)
            vt_sb = pers.tile([128, JT, C], FP8)

            def s_half(nh):
                for jt in range(JT):
                    ps = bigp.tile([128, 512], F32, tag="ps")
                    for k in range(2):
                        nc.tensor.matmul(
                            ps[:],
                            xn_sb[:, 2 * k : 2 * k + 2, jt * 128 : (jt + 1) * 128],
                            t_sb[:, 2 * k : 2 * k + 2, nh * 512 : (nh + 1) * 512],
                            start=(k == 0), stop=(k == 1), perf_mode=DR,
                        )
                    bias = r_sb[:, jt : jt + 1] if with_qbias else 0.0
                    nc.scalar.activation(
                        e_sb[:, jt, nh * 512 : (nh + 1) * 512], ps[:],
                        mybir.ActivationFunctionType.Exp,
                        scale=SCALE, bias=bias,
                    )

            s_half(0)
            for jt in range(JT):
                ps = bigp.tile([128, 512], F32, tag="ps")
                for k in range(2):
                    nc.tensor.matmul(
                        ps[:],
                        xn_sb[:, 2 * k : 2 * k + 2, jt * 128 : (jt + 1) * 128],
                        vw_sb[:, 2 * k : 2 * k + 2, :],
                        start=(k == 0), stop=(k == 1), perf_mode=DR,
                    )
                nc.vector.tensor_copy(vt_sb[:, jt, :], ps[:])
            s_half(1)

            # ---- att^T[i, c] = sum_j E[j, i] v~^T[j, c]; denominators as
            # [128,1] psums; evac = (ps * 1/D) + (x^T + pb), streamed out ----
            rc_sb = pers.tile([128, JT], F32)
            for jt in range(JT):
                dps = smp.tile([128, 1], F32, tag="sps")
                for k in range(4):
                    nc.tensor.matmul(
                        dps[:],
                        e_sb[:, 2 * k : 2 * k + 2, jt * 128 : (jt + 1) * 128],
                        ones2[:],
                        start=(k == 0), stop=(k == 3), perf_mode=DR,
                    )
                nc.vector.reciprocal(rc_sb[:, jt : jt + 1], dps[:])
                ps = bigp.tile([128, 512], F32, tag="ps")
                for k in range(4):
                    nc.tensor.matmul(
                        ps[:],
                        e_sb[:, 2 * k : 2 * k + 2, jt * 128 : (jt + 1) * 128],
                        vt_sb[:, 2 * k : 2 * k + 2, :],
                        start=(k == 0), stop=(k == 3), perf_mode=DR,
                    )
                nc.vector.scalar_tensor_tensor(
                    out=xpb_sb[:, jt, :], in0=ps[:],
                    scalar=rc_sb[:, jt : jt + 1], in1=xpb_sb[:, jt, :],
                    op0=mybir.AluOpType.mult, op1=mybir.AluOpType.add,
                )
                nc.sync.dma_start(y_d[:, jt, :], xpb_sb[:, jt, :])

    nc.compile()
    return nc


def kernel(x, gn_weight, gn_bias, qkv_w, qkv_b, proj_w, proj_b):
    global LAST_RESULTS
    b, c, h, w = x.shape
    assert (b, c, h * w) == (8, C, NPIX)

    f8np = mybir.dt.np(FP8)
    bf16np = mybir.dt.np(BF16)
    x = np.asarray(x, np.float32)
    qkv_b = np.asarray(qkv_b, np.float32)
    qkv_w = np.asarray(qkv_w, np.float32)
    proj_w = np.asarray(proj_w, np.float32)
    # A nonzero q-bias contributes a per-key softmax term r[j] = bq.k_j;
    # k-bias and v-bias fold away (softmax shift invariance / rows sum to 1).
    with_qbias = bool(np.any(qkv_b[0:C]))

    if ("nc", with_qbias) not in _cache:
        _cache[("nc", with_qbias)] = _build(with_qbias)
    nc = _cache[("nc", with_qbias)]

    def col(v):  # [512] vector -> [128, CT] per-partition columns
        return np.ascontiguousarray(np.asarray(v, np.float32).reshape(CT, 128).T)

    def wtile(wT):  # [c_in, cols] -> [128, CT, cols] fp8
        return np.ascontiguousarray(
            np.asarray(wT).reshape(CT, 128, -1).transpose(1, 0, 2).astype(f8np)
        )

    Wq, Wk, Wv = qkv_w[0:C], qkv_w[C : 2 * C], qkv_w[2 * C :]
    M = Wq.astype(np.float64).T @ Wk.astype(np.float64)          # [c_in, c_out]
    WtT = (proj_w.astype(np.float64) @ Wv.astype(np.float64)).T  # [c_in, c_out]
    pb_eff = proj_b + proj_w @ qkv_b[2 * C :]

    shared = {
        "qa": wtile(M),
        "vw": wtile(WtT),
        "gnw": col(gn_weight),
        "gnb": col(gn_bias),
    }
    if with_qbias:
        rw = SCALE * (Wk.astype(np.float64).T @ qkv_b[0:C].astype(np.float64))
        shared["rw"] = np.ascontiguousarray(
            rw.reshape(CT, 128, 1).transpose(1, 0, 2).astype(f8np)
        )

    xs = x.reshape(b, CT, 128, NPIX)
    xt = x.reshape(b, C, NPIX).transpose(0, 2, 1)  # [b, pix, c]
    in_maps = [
        {
            "x": np.ascontiguousarray(xs[i].transpose(1, 0, 2)).astype(bf16np),
            "xpb": np.ascontiguousarray(
                (xt[i] + pb_eff).reshape(JT, 128, C).transpose(1, 0, 2)
            ).astype(np.float32),
            **shared,
        }
        for i in range(b)
    ]

    res = run_bass_kernel_spmd(
        nc, in_maps, core_ids=list(range(8)), trace=TRACE, **TRACE_KW
    )
    LAST_RESULTS = res
    out = np.stack(
        [
            r["y"].transpose(1, 0, 2).reshape(NPIX, C).T.reshape(c, h, w)
            for r in res.results
        ]
    )
    return np.ascontiguousarray(out).astype(np.float32)


# revision 12
# speedup vs baseline: 1.6822x; 1.0224x over previous
"""AttentionBlock (GroupNorm + single-head spatial attention + proj + residual)
on 8 trn2 NeuronCores, data-parallel over the batch (1 image per core).

v4 design (v1 baseline ~94us, v2 ~71us, v3 ~57us):
  - proj_w folded into W_v host-side; q/k folded (t = M^T xn, M = Wq^T Wk).
  - All four big GEMMs (t, S, v~, att) in fp8 e4m3 DoubleRow (2x PE rate,
    measured 216ns per [K=256]x128x512 at full clock).  PSUM stays fp32.
  - Attention output computed transposed (att^T[i,c], pixels on partitions):
    softmax denominators are [128,1] psums from 1-wide matmuls, reciprocal
    is a per-partition scalar in the final evac; residual + proj bias come
    pre-added host-side (xpb = x^T + pb, f32, DMA'd off the critical path).
  - x streams in bf16 (stats + xn only; residual uses exact f32 xpb) in 4
    big chunks (per-chunk DMA overhead ~0.6us dominates small chunks).
  - GroupNorm fully per channel tile (groups never span tiles): each tile's
    stats -> group-combine -> xn completes as its chunk lands, spread over
    gpsimd/vector/scalar so the in-order engine queues don't serialize.
  - Scalar engine runs ONLY Sqrt (early) + Exp: its activation-table reload
    (1.3us) hides in the gn phase.  t/v~ psum evacuations are vector casts.
  - Warm-up: cheap 64-wide bf16 matmuls trickled through the whole gn phase
    keep the HAM clock gate open (PE idle > ~1us drops the PE to half clock
    for the next ~5-12us; v2/v3 lost 4-8us to this).
  - Output y in bf16 (halves the tail DMA; adds ~0.1% quantization, budget
    is 2e-2 and the fp8 path sits at 4.5e-3).
"""

import sys

sys.path.insert(0, "/opt/trn_rl_repo")

import numpy as np

import concourse.bass as bass
import concourse.tile as tile
from concourse import bacc, mybir
from concourse.bass_utils import run_bass_kernel_spmd
from concourse.tile_rust import add_dep_helper

F32 = mybir.dt.float32
BF16 = mybir.dt.bfloat16
FP8 = mybir.dt.float8e4  # e4m3
DR = mybir.MatmulPerfMode.DoubleRow

C = 512          # channels
NPIX = 1024      # pixels per image (32*32)
CT = 4           # channel tiles of 128
JT = 8           # pixel tiles of 128
NH = 2           # halves of NPIX for the 512-wide moving dim
G = 32           # groups
GS = 16          # channels per group
GPT = 8          # groups per channel tile (128/16)
EPS = 1e-5
SCALE = C ** -0.5
WARM0 = 24       # warm-up matmuls at kernel start (~100ns each)
WARMI = 14       # warm-up matmuls interleaved after each channel tile

TRACE = False          # set True (from test.py) to capture an NTFF profile
TRACE_KW = {}          # extra kwargs for run_bass_kernel_spmd
LAST_RESULTS = None    # BassKernelResults of the most recent run

_cache = {}


def _build(with_qbias=False):
    nc = bacc.Bacc("TRN2")

    x_d = nc.dram_tensor("x", [128, CT, NPIX], BF16, kind="ExternalInput")
    xpb_d = nc.dram_tensor("xpb", [128, JT, C], F32, kind="ExternalInput")
    qa_d = nc.dram_tensor("qa", [128, CT, C], FP8, kind="ExternalInput")
    vw_d = nc.dram_tensor("vw", [128, CT, C], FP8, kind="ExternalInput")
    gnw_d = nc.dram_tensor("gnw", [128, CT], F32, kind="ExternalInput")
    gnb_d = nc.dram_tensor("gnb", [128, CT], F32, kind="ExternalInput")
    if with_qbias:
        rw_d = nc.dram_tensor("rw", [128, CT, 1], FP8, kind="ExternalInput")
    y_d = nc.dram_tensor("y", [128, JT, C], BF16, kind="ExternalOutput")

    # Group indicators: within every 128-channel tile the 8 groups are the
    # consecutive 16-channel blocks, identically for each tile.
    ind1 = np.zeros((128, GPT), np.float32)   # group reduce (pre-scaled 1/GS)
    for p in range(128):
        ind1[p, p // GS] = 1.0 / GS
    ind2 = np.zeros((GPT, 128), np.float32)   # broadcast back to channels
    for p in range(128):
        ind2[p // GS, p] = 1.0
    ind1_d = nc.inline_tensor(ind1, name="ind1")
    ind2_d = nc.inline_tensor(ind2, name="ind2")

    with tile.TileContext(nc) as tc:
        with (
            nc.allow_low_precision(reason="fp8 attention path, tol 2e-2"),
            tc.tile_pool(name="persist", bufs=1) as pers,
            tc.tile_pool(name="small", bufs=4) as spool,
            tc.tile_pool(name="bigps", bufs=5, space="PSUM") as bigp,
            tc.tile_pool(name="smallps", bufs=3, space="PSUM") as smp,
        ):
            # ---- constants (no DMA needed) ---------------------------------
            onesc = pers.tile([128, 128], BF16)
            nc.vector.memset(onesc[:], 1.0)
            ones2 = pers.tile([128, 2, 1], FP8)
            nc.vector.memset(ones2[:], 1.0)
            eps8 = pers.tile([GPT, 1], F32)
            nc.vector.memset(eps8[:], EPS)

            warm_ps = bigp.tile([128, 512], F32, tag="ps")

            def warm(n):
                for _ in range(n):
                    nc.tensor.matmul(
                        warm_ps[:, 0:64], onesc[:], onesc[:, 0:64],
                        start=True, stop=True,
                    )

            # ---- x (bf16), one DMA per channel tile ------------------------
            x_sb = pers.tile([128, CT, NPIX], BF16)
            x_dmas = []
            for ct in range(CT):
                x_dmas.append(nc.sync.dma_start(x_sb[:, ct, :], x_d[:, ct, :]))

            # ---- tiny loads ------------------------------------------------
            gnw_sb = pers.tile([128, CT], F32)
            nc.sync.dma_start(gnw_sb[:], gnw_d[:])
            gnb_sb = pers.tile([128, CT], F32)
            nc.sync.dma_start(gnb_sb[:], gnb_d[:])
            ind1_sb = pers.tile([128, GPT], F32)
            nc.sync.dma_start(ind1_sb[:], ind1_d[:])
            ind2_sb = pers.tile([GPT, 128], F32)
            nc.sync.dma_start(ind2_sb[:], ind2_d[:])

            # ---- weights (fp8: 256KB each), serialized behind x ------------
            qa_sb = pers.tile([128, CT, C], FP8)
            d = nc.sync.dma_start(qa_sb[:], qa_d[:])
            add_dep_helper(d.ins, x_dmas[-1].ins, sync=True,
                           reason="x first on the DMA rings")
            vw_sb = pers.tile([128, CT, C], FP8)
            dvw = nc.sync.dma_start(vw_sb[:], vw_d[:])
            add_dep_helper(dvw.ins, x_dmas[-1].ins, sync=True,
                           reason="x first on the DMA rings")
            if with_qbias:
                rw_sb = pers.tile([128, CT, 1], FP8)
                d = nc.sync.dma_start(rw_sb[:], rw_d[:])
                add_dep_helper(d.ins, x_dmas[-1].ins, sync=True,
                               reason="x first on the DMA rings")

            # ---- residual (+proj bias), transposed; needed only at the end -
            xpb_sb = pers.tile([128, JT, C], F32)
            for half in range(4):
                d = nc.sync.dma_start(
                    xpb_sb[:, 2 * half : 2 * half + 2, :],
                    xpb_d[:, 2 * half : 2 * half + 2, :],
                )
                add_dep_helper(d.ins, dvw.ins, sync=True,
                               reason="weights first on the DMA rings")

            warm(WARM0)

            # ---- group norm, fully pipelined per channel tile --------------
            xn_sb = pers.tile([128, CT, NPIX], FP8)
            chA = pers.tile([128, CT], F32)
            chB = pers.tile([128, CT], F32)
            for ct in range(CT):
                st6 = spool.tile([128, 2, 6], F32, tag="st6")
                nc.vector.bn_stats(st6[:, 0, :], x_sb[:, ct, 0:512])
                nc.vector.bn_stats(st6[:, 1, :], x_sb[:, ct, 512:1024])
                mv = spool.tile([128, 2], F32, tag="mv")
                nc.vector.bn_aggr(mv[:], st6[:])
                # statc = [mean, E[x^2]] per channel (sbuf->sbuf: gpsimd)
                statc = spool.tile([128, 2], F32, tag="statc")
                nc.gpsimd.tensor_copy(statc[:, 0:1], mv[:, 0:1])
                nc.gpsimd.tensor_mul(statc[:, 1:2], mv[:, 0:1], mv[:, 0:1])
                nc.gpsimd.tensor_add(statc[:, 1:2], statc[:, 1:2], mv[:, 1:2])
                # group-combine for this tile's 8 groups
                gsp = smp.tile([GPT, 2], F32, tag="sps")
                nc.tensor.matmul(gsp[:], ind1_sb[:], statc[:], start=True, stop=True)
                gs = spool.tile([GPT, 2], F32, tag="gs")
                nc.vector.tensor_copy(gs[:], gsp[:])
                gvar = spool.tile([GPT, 1], F32, tag="gvar")
                nc.gpsimd.tensor_mul(gvar[:], gs[:, 0:1], gs[:, 0:1])
                nc.gpsimd.tensor_sub(gvar[:], gs[:, 1:2], gvar[:])
                grow = spool.tile([GPT, 2], F32, tag="grow")
                gstd = spool.tile([GPT, 1], F32, tag="gstd")
                nc.scalar.activation(
                    gstd[:], gvar[:], mybir.ActivationFunctionType.Sqrt,
                    bias=eps8[:],
                )
                nc.vector.reciprocal(grow[:, 0:1], gstd[:])
                nc.vector.scalar_tensor_tensor(
                    out=grow[:, 1:2], in0=gs[:, 0:1], scalar=-1.0,
                    in1=grow[:, 0:1],
                    op0=mybir.AluOpType.mult, op1=mybir.AluOpType.mult,
                )
                # broadcast to channels; fold gn weight/bias:  xn = x*A + B
                bcp = bigp.tile([128, 2], F32, tag="ps")
                nc.tensor.matmul(bcp[:], ind2_sb[:], grow[:], start=True, stop=True)
                nc.vector.tensor_scalar(
                    out=chA[:, ct : ct + 1], in0=bcp[:, 0:1],
                    scalar1=gnw_sb[:, ct : ct + 1], scalar2=None,
                    op0=mybir.AluOpType.mult,
                )
                nc.vector.scalar_tensor_tensor(
                    out=chB[:, ct : ct + 1], in0=bcp[:, 1:2],
                    scalar=gnw_sb[:, ct : ct + 1], in1=gnb_sb[:, ct : ct + 1],
                    op0=mybir.AluOpType.mult, op1=mybir.AluOpType.add,
                )
                # one 512-half per engine so the tile's xn lands in one step
                for nh, eng in ((0, nc.gpsimd), (1, nc.vector)):
                    eng.tensor_scalar(
                        out=xn_sb[:, ct, nh * 512 : (nh + 1) * 512],
                        in0=x_sb[:, ct, nh * 512 : (nh + 1) * 512],
                        scalar1=chA[:, ct : ct + 1],
                        scalar2=chB[:, ct : ct + 1],
                        op0=mybir.AluOpType.mult,
                        op1=mybir.AluOpType.add,
                    )
                warm(WARMI)

            # ---- t = M^T xn  (fp8 DoubleRow), nh-major for early S ---------
            t_sb = pers.tile([128, CT, NPIX], FP8)
            for nh in range(NH):
                for co in range(CT):
                    ps = bigp.tile([128, 512], F32, tag="ps")
                    for k in range(2):
                        nc.tensor.matmul(
                            ps[:],
                            qa_sb[:, 2 * k : 2 * k + 2, co * 128 : (co + 1) * 128],
                            xn_sb[:, 2 * k : 2 * k + 2, nh * 512 : (nh + 1) * 512],
                            start=(k == 0), stop=(k == 1), perf_mode=DR,
                        )
                    nc.vector.tensor_copy(
                        t_sb[:, co, nh * 512 : (nh + 1) * 512], ps[:]
                    )

            # ---- r[j] = scale * bq . k_j  (only when q-bias nonzero) -------
            if with_qbias:
                r_sb = pers.tile([128, JT], F32)
                for jt in range(JT):
                    rp = smp.tile([128, 1], F32, tag="sps")
                    for k in range(2):
                        nc.tensor.matmul(
                            rp[:],
                            xn_sb[:, 2 * k : 2 * k + 2, jt * 128 : (jt + 1) * 128],
                            rw_sb[:, 2 * k : 2 * k + 2, :],
                            start=(k == 0), stop=(k == 1), perf_mode=DR,
                        )
                    nc.vector.tensor_copy(r_sb[:, jt : jt + 1], rp[:])

            # ---- E[j, i] = exp(scale * S[i, j]); v~ between the nh halves --
            # PE order: S(nh0) -> v~ -> S(nh1) -> att; the v~ matmuls keep the
            # PE busy while the scalar engine works through the nh0 exps.
            e_sb = pers.tile([128, JT, NPIX], FP8)
            vt_sb = pers.tile([128, JT, C], FP8)

            def s_half(nh):
                for jt in range(JT):
                    ps = bigp.tile([128, 512], F32, tag="ps")
                    for k in range(2):
                        nc.tensor.matmul(
                            ps[:],
                            xn_sb[:, 2 * k : 2 * k + 2, jt * 128 : (jt + 1) * 128],
                            t_sb[:, 2 * k : 2 * k + 2, nh * 512 : (nh + 1) * 512],
                            start=(k == 0), stop=(k == 1), perf_mode=DR,
                        )
                    bias = r_sb[:, jt : jt + 1] if with_qbias else 0.0
                    nc.scalar.activation(
                        e_sb[:, jt, nh * 512 : (nh + 1) * 512], ps[:],
                        mybir.ActivationFunctionType.Exp,
                        scale=SCALE, bias=bias,
                    )

            s_half(0)
            for jt in range(JT):
                ps = bigp.tile([128, 512], F32, tag="ps")
                for k in range(2):
                    nc.tensor.matmul(
                        ps[:],
                        xn_sb[:, 2 * k : 2 * k + 2, jt * 128 : (jt + 1) * 128],
                        vw_sb[:, 2 * k : 2 * k + 2, :],
                        start=(k == 0), stop=(k == 1), perf_mode=DR,
                    )
                nc.vector.tensor_copy(vt_sb[:, jt, :], ps[:])
            s_half(1)

            # ---- att^T[i, c] = sum_j E[j, i] v~^T[j, c]; denominators as
            # [128,1] psums; evac = (ps * 1/D) + (x^T + pb), streamed out ----
            rc_sb = pers.tile([128, JT], F32)
            y_sb = pers.tile([128, JT, C], BF16)
            for jt in range(JT):
                dps = smp.tile([128, 1], F32, tag="sps")
                for k in range(4):
                    nc.tensor.matmul(
                        dps[:],
                        e_sb[:, 2 * k : 2 * k + 2, jt * 128 : (jt + 1) * 128],
                        ones2[:],
                        start=(k == 0), stop=(k == 3), perf_mode=DR,
                    )
                nc.vector.reciprocal(rc_sb[:, jt : jt + 1], dps[:])
                ps = bigp.tile([128, 512], F32, tag="ps")
                for k in range(4):
                    nc.tensor.matmul(
                        ps[:],
                        e_sb[:, 2 * k : 2 * k + 2, jt * 128 : (jt + 1) * 128],
                        vt_sb[:, 2 * k : 2 * k + 2, :],
                        start=(k == 0), stop=(k == 3), perf_mode=DR,
                    )
                nc.vector.scalar_tensor_tensor(
                    out=y_sb[:, jt, :], in0=ps[:],
                    scalar=rc_sb[:, jt : jt + 1], in1=xpb_sb[:, jt, :],
                    op0=mybir.AluOpType.mult, op1=mybir.AluOpType.add,
                )
                nc.sync.dma_start(y_d[:, jt, :], y_sb[:, jt, :])

    nc.compile()
    return nc


def kernel(x, gn_weight, gn_bias, qkv_w, qkv_b, proj_w, proj_b):
    global LAST_RESULTS
    b, c, h, w = x.shape
    assert (b, c, h * w) == (8, C, NPIX)

    f8np = mybir.dt.np(FP8)
    bf16np = mybir.dt.np(BF16)
    x = np.asarray(x, np.float32)
    qkv_b = np.asarray(qkv_b, np.float32)
    qkv_w = np.asarray(qkv_w, np.float32)
    proj_w = np.asarray(proj_w, np.float32)
    # A nonzero q-bias contributes a per-key softmax term r[j] = bq.k_j;
    # k-bias and v-bias fold away (softmax shift invariance / rows sum to 1).
    with_qbias = bool(np.any(qkv_b[0:C]))

    if ("nc", with_qbias) not in _cache:
        _cache[("nc", with_qbias)] = _build(with_qbias)
    nc = _cache[("nc", with_qbias)]

    def col(v):  # [512] vector -> [128, CT] per-partition columns
        return np.ascontiguousarray(np.asarray(v, np.float32).reshape(CT, 128).T)

    def wtile(wT):  # [c_in, cols] -> [128, CT, cols] fp8
        return np.ascontiguousarray(
            np.asarray(wT).reshape(CT, 128, -1).transpose(1, 0, 2).astype(f8np)
        )

    Wq, Wk, Wv = qkv_w[0:C], qkv_w[C : 2 * C], qkv_w[2 * C :]
    M = Wq.astype(np.float64).T @ Wk.astype(np.float64)          # [c_in, c_out]
    WtT = (proj_w.astype(np.float64) @ Wv.astype(np.float64)).T  # [c_in, c_out]
    pb_eff = proj_b + proj_w @ qkv_b[2 * C :]

    shared = {
        "qa": wtile(M),
        "vw": wtile(WtT),
        "gnw": col(gn_weight),
        "gnb": col(gn_bias),
    }
    if with_qbias:
        rw = SCALE * (Wk.astype(np.float64).T @ qkv_b[0:C].astype(np.float64))
        shared["rw"] = np.ascontiguousarray(
            rw.reshape(CT, 128, 1).transpose(1, 0, 2).astype(f8np)
        )

    xs = x.reshape(b, CT, 128, NPIX)
    xt = x.reshape(b, C, NPIX).transpose(0, 2, 1)  # [b, pix, c]
    in_maps = [
        {
            "x": np.ascontiguousarray(xs[i].transpose(1, 0, 2)).astype(bf16np),
            "xpb": np.ascontiguousarray(
                (xt[i] + pb_eff).reshape(JT, 128, C).transpose(1, 0, 2)
            ).astype(np.float32),
            **shared,
        }
        for i in range(b)
    ]

    res = run_bass_kernel_spmd(
        nc, in_maps, core_ids=list(range(8)), trace=TRACE, **TRACE_KW
    )
    LAST_RESULTS = res
    out = np.stack(
        [
            r["y"].astype(np.float32).transpose(1, 0, 2)
            .reshape(NPIX, C).T.reshape(c, h, w)
            for r in res.results
        ]
    )
    return np.ascontiguousarray(out).astype(np.float32)
